# revision 16
# baseline (speedup 1.0000x reference)
"""Trainium2 Bass kernel for nn_MinibatchLayer (pairwise L1 minibatch-discrimination layer).

Math (reference):
    norm   = sqrt(sum(theta^2, axis=0))                      # [K,P]
    kernel = theta * (exp(lws)/norm)                         # [D,K,P]
    actv   = einsum('bd,dkp->bkp', x, kernel)                # [B,K,P]
    M[i,k,j] = sum_p |actv[i,k,p] - actv[j,k,p]|  (+1e6 on diag)
    f      = sum_j exp(-M) + bias                            # [B,K]
    out    = concat([x, f], axis=1)                          # [B,D+K]

Distribution: shard K=128 across 8 cores (16 kernels/core). Each core reads the
full x (transposed, bf16) + its theta slice; no collectives. Host assembles
out = [x | f] from per-core f blocks.

Per-core pipeline, using |d| = 2*relu(d) - d and the SYMMETRY of
A[i,j] = exp(-sum_p |a_i - a_j|): only upper-triangle tiles are computed.
For i-chunk c (128 i's on partitions) the j range is [128c, 512), length
L = 512-128c; tiles (c,k) for k in 0..15:
  - R[p] = relu(Y[p] - a_i)      (VectorE tensor_scalar sub+max, bf16 4x mode)
  - PSUM = sum_p R[p] - 0.5*sum_p Y[p]   (6 TensorE identity-matmul accums)
  - E    = exp(-2*PSUM - sum_p a_i) = A[i, j-range]  (ScalarE, with accum_out
           giving the ROW sums -> credits f[i] for all pairs in the tile,
           including the diagonal block; exp(0)=1 diag removed via bias-1)
  - COLUMN sums over the tile's partitions, excluding the diagonal block,
    credit f[j] for j > chunk c: one PE matmul per tile with a one-hot-row
    lhsT ([128,16], column k all-ones) puts the colsum into ROW k of a per-c
    PSUM collector, accumulated across k via the PSUM accumulate bit.
  - End: collector pieces are summed, PE-transposed to columns, and added to
    the accum_out partials + (bias - 1); fout[B, KC] per core.
"""

import os
import numpy as np

B, D, K, P = 512, 2048, 128, 5
N_CORES = 8
KC = K // N_CORES          # 16 kernels per core
BT = KC * P                # 80 (k,p) columns per core
NB = B // 128              # 4 batch chunks
ND = D // 128              # 16 contraction chunks

_cache = {}


def _build():
    from concourse import bacc, tile, mybir

    dt = mybir.dt
    f32, bf16 = dt.float32, dt.bfloat16
    Alu = mybir.AluOpType
    Act = mybir.ActivationFunctionType

    nc = bacc.Bacc("TRN2", target_bir_lowering=False, debug=False,
                   num_devices=N_CORES)

    xT = nc.dram_tensor("xT", [128, ND * B], bf16, kind="ExternalInput").ap()
    theta = nc.dram_tensor("theta", [128, ND * BT], bf16, kind="ExternalInput").ap()
    lws = nc.dram_tensor("lws", [1, BT], f32, kind="ExternalInput").ap()
    biasc = nc.dram_tensor("biasc", [1, KC], f32, kind="ExternalInput").ap()
    identin = nc.dram_tensor("identin", [128, 128], bf16, kind="ExternalInput").ap()
    pselin = nc.dram_tensor("pselin", [BT, KC], bf16, kind="ExternalInput").ap()
    fout = nc.dram_tensor("fout", [B, KC], f32, kind="ExternalOutput").ap()
    avT_dram = nc.dram_tensor("avT_dram", [BT + KC + 1, B], bf16).ap()
    kcolsin = nc.dram_tensor("kcolsin", [128, KC * KC], bf16,
                             kind="ExternalInput").ap()

    with tile.TileContext(nc) as tc:
        with (
            tc.tile_pool(name="const", bufs=1) as constp,
            tc.tile_pool(name="stage", bufs=2) as stagep,
            tc.tile_pool(name="ps", bufs=2, space="PSUM") as psp,
            tc.tile_pool(name="mps", bufs=3, space="PSUM") as mpsp,
            tc.tile_pool(name="cps", bufs=2, space="PSUM") as cpsp,
            tc.tile_pool(name="db", bufs=3) as dbp,
            tc.tile_pool(name="g", bufs=4) as gp,
        ):
            # ---- constants ----
            identb = constp.tile([128, 128], bf16, tag="identb")
            nc.scalar.dma_start(out=identb[:], in_=identin[:])
            ones_bf = constp.tile([128, 1], bf16, tag="ones_bf")
            nc.vector.memset(ones_bf[:], 1.0)
            ones_f1 = constp.tile([1, 1], f32, tag="ones_f1")
            nc.vector.memset(ones_f1[:], 1.0)
            ones_row = constp.tile([1, 128], f32, tag="ones_row")
            nc.vector.memset(ones_row[:], 1.0)
            psel = constp.tile([BT, KC], bf16, tag="psel")
            nc.scalar.dma_start(out=psel[:], in_=pselin[:])
            # one-hot-row colsum weights: kcols[:, k*KC+m] = (m == k)
            kcols = constp.tile([128, KC * KC], bf16, tag="kcols")
            nc.scalar.dma_start(out=kcols[:], in_=kcolsin[:])
            identf = constp.tile([KC, KC], f32, tag="identf")
            nc.scalar.activation(identf[:], identb[0:KC, 0:KC], Act.Copy)

            # ---- load x^T and theta in two big DMAs ----
            thtile = constp.tile([128, ND * BT], bf16, tag="thtile")
            xtile = constp.tile([128, ND * B], bf16, tag="xtile")
            for q in range(4):
                w = ND * BT // 4
                nc.scalar.dma_start(out=thtile[:, q * w:(q + 1) * w],
                                  in_=theta[:, q * w:(q + 1) * w])
            for q in range(4):
                w = ND * B // 4
                nc.sync.dma_start(out=xtile[:, q * w:(q + 1) * w],
                                  in_=xT[:, q * w:(q + 1) * w])
            xts = [xtile[:, c * B:(c + 1) * B] for c in range(ND)]
            ths = [thtile[:, c * BT:(c + 1) * BT] for c in range(ND)]

            # ---- norm^2 = sum_d theta^2 (per (k,p)) ----
            n2ps = psp.tile([1, BT], f32, tag="ph1")
            for c in range(ND):
                sq = stagep.tile([128, BT], bf16, tag="sq")
                nc.vector.tensor_mul(sq[:], ths[c], ths[c])
                nc.tensor.matmul(n2ps[:], lhsT=ones_bf[:], rhs=sq[:],
                                 start=(c == 0), stop=(c == ND - 1))

            # ---- scale = exp(lws - 0.5*ln(norm^2)), as a [BT,1] column ----
            lnrow = stagep.tile([1, BT], f32, tag="lnrow")
            nc.scalar.activation(lnrow[:], n2ps[:], Act.Ln)
            lwsrow = stagep.tile([1, BT], f32, tag="lwsrow")
            nc.scalar.dma_start(out=lwsrow[:], in_=lws[:])
            arg = stagep.tile([1, BT], f32, tag="arg")
            nc.vector.tensor_scalar(out=arg[:], in0=lnrow[:], scalar1=-0.5,
                                    scalar2=None, op0=Alu.mult)
            nc.vector.tensor_add(arg[:], arg[:], lwsrow[:])
            argT_ps = psp.tile([BT, 1], f32, tag="ph1")
            nc.tensor.transpose(argT_ps[:], arg[:], ones_f1[:])
            scale_col = stagep.tile([BT, 1], f32, tag="scale_col")
            nc.scalar.activation(scale_col[:], argT_ps[:], Act.Exp)

            # ---- actv_T = (theta.T @ x.T) * scale  -> bf16 [80, 512] ----
            avT_ps = psp.tile([BT, B], f32, tag="ph1")
            for c in range(ND):
                nc.tensor.matmul(avT_ps[:], lhsT=ths[c], rhs=xts[c],
                                 start=(c == 0), stop=(c == ND - 1))
            avT_bf = constp.tile([BT, B], bf16, tag="avT_bf")
            nc.scalar.activation(avT_bf[:], avT_ps[:], Act.Copy,
                                 scale=scale_col[:])
            nc.sync.dma_start(out=avT_dram[0:BT, :], in_=avT_bf[:])
            yall = constp.tile([128, BT * B], bf16, tag="yall")
            ysn_all = constp.tile([128, KC * B], bf16, tag="ysn_all")
            NQ, QR = 8, BT // 8

            def yall_bcast(q):
                nc.sync.dma_start(
                    out=yall[:, q * QR * B:(q + 1) * QR * B].rearrange(
                        "a (b c) -> a b c", b=QR),
                    in_=avT_dram[q * QR:(q + 1) * QR, :].partition_broadcast(128))

            yall_bcast(0)
            ysn_ps = psp.tile([KC, B], f32, tag="ph1")
            nc.tensor.matmul(ysn_ps[:], lhsT=psel[:], rhs=avT_bf[:])
            ysn_sb = stagep.tile([KC, B], bf16, tag="ysn_sb")
            nc.scalar.activation(ysn_sb[:], ysn_ps[:], Act.Copy, scale=-0.5)
            nc.sync.dma_start(out=avT_dram[BT:BT + KC, :], in_=ysn_sb[:])
            for h in range(2):
                nc.sync.dma_start(
                    out=ysn_all[:, h * 8 * B:(h + 1) * 8 * B].rearrange(
                        "a (b c) -> a b c", b=8),
                    in_=avT_dram[BT + h * 8:BT + (h + 1) * 8,
                                 :].partition_broadcast(128))
            for q in range(1, NQ):
                yall_bcast(q)

            # keep TensorE busy across the broadcast wait so HAM stays warm
            warm_ps = psp.tile([128, B], f32, tag="ph1")
            for w in range(24):
                nc.tensor.matmul(warm_ps[:], lhsT=identb[:],
                                 rhs=xts[w % ND],
                                 start=(w == 0), stop=(w == 23))
            warm_sb = stagep.tile([1, 1], bf16, tag="warm_sb")
            nc.scalar.activation(warm_sb[:], warm_ps[0:1, 0:1], Act.Copy)
            nc.sync.dma_start(out=avT_dram[BT + KC:BT + KC + 1, 0:1], in_=warm_sb[:])

            # ---- actv (b-major) bf16 via PE transpose: 4 x [128, 80] ----
            avs = []
            for bc in range(NB):
                av_ps = psp.tile([128, BT], bf16, tag="ph1")
                nc.tensor.transpose(av_ps[:], avT_bf[:, bc * 128:(bc + 1) * 128],
                                    identb[0:BT, 0:BT])
                av = constp.tile([128, BT], f32, tag=f"av{bc}")
                nc.scalar.activation(av[:], av_ps[:], Act.Copy)
                avs.append(av)

            # ---- bias tile: [128, KC] = bias - 1 (diagonal correction) ----
            brow = stagep.tile([1, KC], f32, tag="brow")
            nc.scalar.dma_start(out=brow[:], in_=biasc[:])
            bm1 = stagep.tile([1, KC], f32, tag="bm1")
            nc.vector.tensor_scalar(out=bm1[:], in0=brow[:], scalar1=1.0,
                                    scalar2=None, op0=Alu.subtract)
            bps = psp.tile([128, KC], f32, tag="ph1")
            nc.tensor.matmul(bps[:], lhsT=ones_row[:], rhs=bm1[:])
            bbias = constp.tile([128, KC], f32, tag="bbias")
            nc.scalar.activation(bbias[:], bps[:], Act.Copy)

            # ---- negated per-(i,k) sums over p (Exp bias columns) ----
            nsas = []
            for bc in range(NB):
                nsa = constp.tile([128, KC], f32, tag=f"nsa{bc}", name=f"nsa{bc}")
                nc.vector.tensor_reduce(
                    nsa[:], avs[bc][:].rearrange("a (b c) -> a b c", c=P),
                    axis=mybir.AxisListType.X, op=Alu.add, negate=True)
                nsas.append(nsa)

            # ---- f row-sum accumulators (Act accum_out partials) ----
            fsbs = [constp.tile([128, KC], f32, tag=f"fsb{bc}", name=f"fsb{bc}")
                    for bc in range(NB)]
            # ---- column-sum collectors (SBUF copies): k rows, free j ----
            colls = [constp.tile([KC, 384 - 128 * c], f32, tag=f"coll{c}",
                                 name=f"coll{c}") for c in range(3)]

            # ---- main loop: upper-triangle tiles (c, k); c outer ----
            # The colsum matmul for tile k reads Act's E output; emit it with
            # a 2-tile lag so the in-order PE never stalls on ScalarE.
            COL_LAG = 2
            pending = []  # (c, k, coll_ps, etile, L)

            def emit_colsum(c, k, coll_ps, etile, L):
                nc.tensor.matmul(
                    coll_ps[:, 0:L - 128],
                    lhsT=kcols[:, k * KC:(k + 1) * KC],
                    rhs=etile[:, 128:L],
                    start=(k == 0), stop=(k == KC - 1),
                    skip_group_check=True)

            for c in range(NB):
                j0 = 128 * c
                L = B - j0
                coll_ps = (cpsp.tile([KC, 384], f32, tag="coll",
                                     name=f"coll_ps{c}") if c < 3 else None)
                for k in range(KC):
                    dtile = dbp.tile([128, P * B], bf16, tag="d")
                    m_ps = mpsp.tile([128, B], f32, tag="m")
                    for p in range(P):
                        ys = yall[:, (k * P + p) * B + j0:(k * P + p + 1) * B]
                        dsl = dtile[:, p * L:(p + 1) * L]
                        nc.vector.tensor_scalar(
                            out=dsl, in0=ys,
                            scalar1=avs[c][:, k * P + p:k * P + p + 1],
                            scalar2=0.0, op0=Alu.subtract, op1=Alu.max)
                        nc.tensor.matmul(m_ps[:, 0:L], lhsT=identb[:], rhs=dsl,
                                         start=(p == 0), stop=False)
                    nc.tensor.matmul(m_ps[:, 0:L], lhsT=identb[:],
                                     rhs=ysn_all[:, k * B + j0:(k + 1) * B],
                                     start=False, stop=True)
                    etile = gp.tile([128, B], bf16, tag="g")
                    nc.scalar.activation(etile[:, 0:L], m_ps[:, 0:L], Act.Exp,
                                         scale=-2.0,
                                         bias=nsas[c][:, k:k + 1],
                                         accum_out=fsbs[c][:, k:k + 1])
                    if c < 3:
                        pending.append((c, k, coll_ps, etile, L))
                        if len(pending) > COL_LAG:
                            emit_colsum(*pending.pop(0))
                # flush this chunk's lagged colsums before copying out
                if c < 3:
                    while pending:
                        emit_colsum(*pending.pop(0))
                    nc.scalar.activation(colls[c][:], coll_ps[:, 0:384 - 128 * c],
                                         Act.Copy)

            # ---- assemble fout: rowsums + bias-1 + transposed colsum pieces ----
            for c in range(NB):
                of = gp.tile([128, KC], f32, tag="of")
                nc.vector.tensor_add(of[:], fsbs[c][:], bbias[:])
                if c > 0:
                    # colsum pieces for output chunk c from collectors c' < c
                    acc16 = stagep.tile([KC, 128], f32, tag="acc16")
                    first = True
                    for cp in range(c):
                        off = 128 * (c - cp - 1)
                        piece = colls[cp][:, off:off + 128]
                        if first:
                            nc.vector.tensor_copy(acc16[:], piece)
                            first = False
                        else:
                            nc.vector.tensor_add(acc16[:], acc16[:], piece)
                    tp_ps = psp.tile([128, KC], f32, tag="ph1")
                    nc.tensor.transpose(tp_ps[:], acc16[:], identf[:])
                    nc.vector.tensor_add(of[:], of[:], tp_ps[:])
                nc.sync.dma_start(out=fout[c * 128:(c + 1) * 128, :], in_=of[:])

    nc.compile()
    return nc


def _get_program():
    if "nc" not in _cache:
        _cache["nc"] = _build()
    return _cache["nc"]


def kernel(x, theta, log_weight_scale, bias, _trace=False):
    import ml_dtypes
    from concourse.bass_utils import run_bass_kernel_spmd

    x = np.asarray(x, dtype=np.float32)
    theta = np.asarray(theta, dtype=np.float32)
    log_weight_scale = np.asarray(log_weight_scale, dtype=np.float32)
    bias = np.asarray(bias, dtype=np.float32)

    nc = _get_program()

    bf = ml_dtypes.bfloat16
    xTl = np.ascontiguousarray(
        x.T.reshape(ND, 128, B).transpose(1, 0, 2).reshape(128, ND * B)
    ).astype(bf)
    ident = np.eye(128, dtype=np.float32).astype(bf)
    # block selector: row (k,p) -> column k (for per-k sums over p)
    psel = np.repeat(np.eye(KC, dtype=np.float32), P, axis=0).astype(bf)
    # one-hot-row colsum weights: kcols[:, k*KC+m] = (m == k)
    kc_np = np.zeros((128, KC * KC), dtype=np.float32)
    for k_ in range(KC):
        kc_np[:, k_ * KC + k_] = 1.0
    kc_np = kc_np.astype(bf)

    in_maps = []
    for c in range(N_CORES):
        ks = slice(c * KC, (c + 1) * KC)
        th = np.ascontiguousarray(
            theta[:, ks, :].reshape(ND, 128, BT)
            .transpose(1, 0, 2).reshape(128, ND * BT)).astype(bf)
        lw = np.ascontiguousarray(
            log_weight_scale[ks, :].reshape(1, BT)).astype(np.float32)
        bi = np.ascontiguousarray(bias[ks].reshape(1, KC)).astype(np.float32)
        in_maps.append({"xT": xTl, "theta": th, "lws": lw, "biasc": bi,
                        "identin": ident, "pselin": psel, "kcolsin": kc_np})

    res = run_bass_kernel_spmd(nc, in_maps, list(range(N_CORES)),
                               trace=bool(_trace))
    f = np.concatenate([res.results[c]["fout"] for c in range(N_CORES)], axis=1)
    out = np.concatenate([x, f.astype(np.float32)], axis=1)
    if _trace:
        return out, res
    return out


# revision 18
# speedup vs baseline: 1.0196x; 1.0196x over previous
"""Trainium2 Bass kernel for nn_MinibatchLayer (pairwise L1 minibatch-discrimination layer).

Math (reference):
    norm   = sqrt(sum(theta^2, axis=0))                      # [K,P]
    kernel = theta * (exp(lws)/norm)                         # [D,K,P]
    actv   = einsum('bd,dkp->bkp', x, kernel)                # [B,K,P]
    M[i,k,j] = sum_p |actv[i,k,p] - actv[j,k,p]|  (+1e6 on diag)
    f      = sum_j exp(-M) + bias                            # [B,K]
    out    = concat([x, f], axis=1)                          # [B,D+K]

Distribution: shard K=128 across 8 cores (16 kernels/core). Each core reads the
full x (transposed, bf16) + its theta slice; no collectives. Host assembles
out = [x | f] from per-core f blocks.

Per-core pipeline, using |d| = 2*relu(d) - d and the SYMMETRY of
A[i,j] = exp(-sum_p |a_i - a_j|): only upper-triangle tiles are computed.
For i-chunk c (128 i's on partitions) the j range is [128c, 512), length
L = 512-128c; tiles (c,k) for k in 0..15:
  - R[p] = relu(Y[p] - a_i)      (VectorE tensor_scalar sub+max, bf16 4x mode)
  - PSUM = sum_p R[p] - 0.5*sum_p Y[p]   (6 TensorE identity-matmul accums)
  - E    = exp(-2*PSUM - sum_p a_i) = A[i, j-range]  (ScalarE, with accum_out
           giving the ROW sums -> credits f[i] for all pairs in the tile,
           including the diagonal block; exp(0)=1 diag removed via bias-1)
  - COLUMN sums over the tile's partitions, excluding the diagonal block,
    credit f[j] for j > chunk c: one PE matmul per tile with a one-hot-row
    lhsT ([128,16], column k all-ones) puts the colsum into ROW k of a per-c
    PSUM collector, accumulated across k via the PSUM accumulate bit.
  - End: collector pieces are summed, PE-transposed to columns, and added to
    the accum_out partials + (bias - 1); fout[B, KC] per core.
"""

import os
import numpy as np

B, D, K, P = 512, 2048, 128, 5
N_CORES = 8
KC = K // N_CORES          # 16 kernels per core
BT = KC * P                # 80 (k,p) columns per core
NB = B // 128              # 4 batch chunks
ND = D // 128              # 16 contraction chunks

_cache = {}


def _build():
    from concourse import bacc, tile, mybir

    dt = mybir.dt
    f32, bf16 = dt.float32, dt.bfloat16
    Alu = mybir.AluOpType
    Act = mybir.ActivationFunctionType

    nc = bacc.Bacc("TRN2", target_bir_lowering=False, debug=False,
                   num_devices=N_CORES)

    xT = nc.dram_tensor("xT", [128, ND * B], bf16, kind="ExternalInput").ap()
    theta = nc.dram_tensor("theta", [128, ND * BT], bf16, kind="ExternalInput").ap()
    lws = nc.dram_tensor("lws", [1, BT], f32, kind="ExternalInput").ap()
    biasc = nc.dram_tensor("biasc", [1, KC], f32, kind="ExternalInput").ap()
    identin = nc.dram_tensor("identin", [128, 128], bf16, kind="ExternalInput").ap()
    pselin = nc.dram_tensor("pselin", [BT, KC], bf16, kind="ExternalInput").ap()
    fout = nc.dram_tensor("fout", [B, KC], f32, kind="ExternalOutput").ap()
    avT_dram = nc.dram_tensor("avT_dram", [BT + KC + 1, B], bf16).ap()
    kcolsin = nc.dram_tensor("kcolsin", [128, KC * KC], bf16,
                             kind="ExternalInput").ap()

    with tile.TileContext(nc) as tc:
        with (
            tc.tile_pool(name="const", bufs=1) as constp,
            tc.tile_pool(name="stage", bufs=2) as stagep,
            tc.tile_pool(name="ps", bufs=2, space="PSUM") as psp,
            tc.tile_pool(name="mps", bufs=3, space="PSUM") as mpsp,
            tc.tile_pool(name="cps", bufs=2, space="PSUM") as cpsp,
            tc.tile_pool(name="db", bufs=3) as dbp,
            tc.tile_pool(name="g", bufs=18) as gp,
        ):
            # ---- constants ----
            identb = constp.tile([128, 128], bf16, tag="identb")
            nc.scalar.dma_start(out=identb[:], in_=identin[:])
            ones_bf = constp.tile([128, 1], bf16, tag="ones_bf")
            nc.vector.memset(ones_bf[:], 1.0)
            ones_f1 = constp.tile([1, 1], f32, tag="ones_f1")
            nc.vector.memset(ones_f1[:], 1.0)
            ones_row = constp.tile([1, 128], f32, tag="ones_row")
            nc.vector.memset(ones_row[:], 1.0)
            psel = constp.tile([BT, KC], bf16, tag="psel")
            nc.scalar.dma_start(out=psel[:], in_=pselin[:])
            # one-hot-row colsum weights: kcols[:, k*KC+m] = (m == k)
            kcols = constp.tile([128, KC * KC], bf16, tag="kcols")
            nc.scalar.dma_start(out=kcols[:], in_=kcolsin[:])
            identf = constp.tile([KC, KC], f32, tag="identf")
            nc.scalar.activation(identf[:], identb[0:KC, 0:KC], Act.Copy)

            # ---- load x^T and theta in two big DMAs ----
            thtile = constp.tile([128, ND * BT], bf16, tag="thtile")
            xtile = constp.tile([128, ND * B], bf16, tag="xtile")
            for q in range(4):
                w = ND * BT // 4
                nc.scalar.dma_start(out=thtile[:, q * w:(q + 1) * w],
                                  in_=theta[:, q * w:(q + 1) * w])
            for q in range(4):
                w = ND * B // 4
                nc.sync.dma_start(out=xtile[:, q * w:(q + 1) * w],
                                  in_=xT[:, q * w:(q + 1) * w])
            xts = [xtile[:, c * B:(c + 1) * B] for c in range(ND)]
            ths = [thtile[:, c * BT:(c + 1) * BT] for c in range(ND)]

            # ---- norm^2 = sum_d theta^2 (per (k,p)) ----
            n2ps = psp.tile([1, BT], f32, tag="ph1")
            for c in range(ND):
                sq = stagep.tile([128, BT], bf16, tag="sq")
                nc.vector.tensor_mul(sq[:], ths[c], ths[c])
                nc.tensor.matmul(n2ps[:], lhsT=ones_bf[:], rhs=sq[:],
                                 start=(c == 0), stop=(c == ND - 1))

            # ---- scale = exp(lws - 0.5*ln(norm^2)), as a [BT,1] column ----
            lnrow = stagep.tile([1, BT], f32, tag="lnrow")
            nc.scalar.activation(lnrow[:], n2ps[:], Act.Ln)
            lwsrow = stagep.tile([1, BT], f32, tag="lwsrow")
            nc.scalar.dma_start(out=lwsrow[:], in_=lws[:])
            arg = stagep.tile([1, BT], f32, tag="arg")
            nc.vector.tensor_scalar(out=arg[:], in0=lnrow[:], scalar1=-0.5,
                                    scalar2=None, op0=Alu.mult)
            nc.vector.tensor_add(arg[:], arg[:], lwsrow[:])
            argT_ps = psp.tile([BT, 1], f32, tag="ph1")
            nc.tensor.transpose(argT_ps[:], arg[:], ones_f1[:])
            scale_col = stagep.tile([BT, 1], f32, tag="scale_col")
            nc.scalar.activation(scale_col[:], argT_ps[:], Act.Exp)

            # ---- actv_T = (theta.T @ x.T) * scale  -> bf16 [80, 512] ----
            avT_ps = psp.tile([BT, B], f32, tag="ph1")
            for c in range(ND):
                nc.tensor.matmul(avT_ps[:], lhsT=ths[c], rhs=xts[c],
                                 start=(c == 0), stop=(c == ND - 1))
            avT_bf = constp.tile([BT, B], bf16, tag="avT_bf")
            nc.scalar.activation(avT_bf[:], avT_ps[:], Act.Copy,
                                 scale=scale_col[:])
            nc.sync.dma_start(out=avT_dram[0:BT, :], in_=avT_bf[:])
            yall = constp.tile([128, BT * B], bf16, tag="yall")
            ysn_all = constp.tile([128, KC * B], bf16, tag="ysn_all")
            NQ, QR = 8, BT // 8

            def yall_bcast(q):
                nc.sync.dma_start(
                    out=yall[:, q * QR * B:(q + 1) * QR * B].rearrange(
                        "a (b c) -> a b c", b=QR),
                    in_=avT_dram[q * QR:(q + 1) * QR, :].partition_broadcast(128))

            yall_bcast(0)
            ysn_ps = psp.tile([KC, B], f32, tag="ph1")
            nc.tensor.matmul(ysn_ps[:], lhsT=psel[:], rhs=avT_bf[:])
            ysn_sb = stagep.tile([KC, B], bf16, tag="ysn_sb")
            nc.scalar.activation(ysn_sb[:], ysn_ps[:], Act.Copy, scale=-0.5)
            nc.sync.dma_start(out=avT_dram[BT:BT + KC, :], in_=ysn_sb[:])
            for h in range(2):
                nc.sync.dma_start(
                    out=ysn_all[:, h * 8 * B:(h + 1) * 8 * B].rearrange(
                        "a (b c) -> a b c", b=8),
                    in_=avT_dram[BT + h * 8:BT + (h + 1) * 8,
                                 :].partition_broadcast(128))
            for q in range(1, NQ):
                yall_bcast(q)

            # keep TensorE busy across the broadcast wait so HAM stays warm
            warm_ps = psp.tile([128, B], f32, tag="ph1")
            for w in range(24):
                nc.tensor.matmul(warm_ps[:], lhsT=identb[:],
                                 rhs=xts[w % ND],
                                 start=(w == 0), stop=(w == 23))
            warm_sb = stagep.tile([1, 1], bf16, tag="warm_sb")
            nc.scalar.activation(warm_sb[:], warm_ps[0:1, 0:1], Act.Copy)
            nc.sync.dma_start(out=avT_dram[BT + KC:BT + KC + 1, 0:1], in_=warm_sb[:])

            # ---- actv (b-major) bf16 via PE transpose: 4 x [128, 80] ----
            avs = []
            for bc in range(NB):
                av_ps = psp.tile([128, BT], bf16, tag="ph1")
                nc.tensor.transpose(av_ps[:], avT_bf[:, bc * 128:(bc + 1) * 128],
                                    identb[0:BT, 0:BT])
                av = constp.tile([128, BT], f32, tag=f"av{bc}")
                nc.scalar.activation(av[:], av_ps[:], Act.Copy)
                avs.append(av)

            # ---- bias tile: [128, KC] = bias - 1 (diagonal correction) ----
            brow = stagep.tile([1, KC], f32, tag="brow")
            nc.scalar.dma_start(out=brow[:], in_=biasc[:])
            bm1 = stagep.tile([1, KC], f32, tag="bm1")
            nc.vector.tensor_scalar(out=bm1[:], in0=brow[:], scalar1=1.0,
                                    scalar2=None, op0=Alu.subtract)
            bps = psp.tile([128, KC], f32, tag="ph1")
            nc.tensor.matmul(bps[:], lhsT=ones_row[:], rhs=bm1[:])
            bbias = constp.tile([128, KC], f32, tag="bbias")
            nc.scalar.activation(bbias[:], bps[:], Act.Copy)

            # ---- negated per-(i,k) sums over p (Exp bias columns) ----
            nsas = []
            for bc in range(NB):
                nsa = constp.tile([128, KC], f32, tag=f"nsa{bc}", name=f"nsa{bc}")
                nc.vector.tensor_reduce(
                    nsa[:], avs[bc][:].rearrange("a (b c) -> a b c", c=P),
                    axis=mybir.AxisListType.X, op=Alu.add, negate=True)
                nsas.append(nsa)

            # ---- f row-sum accumulators (Act accum_out partials) ----
            fsbs = [constp.tile([128, KC], f32, tag=f"fsb{bc}", name=f"fsb{bc}")
                    for bc in range(NB)]
            # ---- column-sum collectors (SBUF copies): k rows, free j ----
            colls = [constp.tile([KC, 384 - 128 * c], f32, tag=f"coll{c}",
                                 name=f"coll{c}") for c in range(3)]

            # ---- main loop: upper-triangle tiles (c, k); c outer ----
            # Colsum matmuls read Act's E output; batching them at the END of
            # each chunk's k-loop keeps the in-order PE stream free of
            # cross-engine backedges (all 16 E tiles stay alive per chunk).
            for c in range(NB):
                j0 = 128 * c
                L = B - j0
                etiles = []
                for k in range(KC):
                    dtile = dbp.tile([128, P * B], bf16, tag="d")
                    m_ps = mpsp.tile([128, B], f32, tag="m")
                    for p in range(P):
                        ys = yall[:, (k * P + p) * B + j0:(k * P + p + 1) * B]
                        dsl = dtile[:, p * L:(p + 1) * L]
                        nc.vector.tensor_scalar(
                            out=dsl, in0=ys,
                            scalar1=avs[c][:, k * P + p:k * P + p + 1],
                            scalar2=0.0, op0=Alu.subtract, op1=Alu.max)
                        nc.tensor.matmul(m_ps[:, 0:L], lhsT=identb[:], rhs=dsl,
                                         start=(p == 0), stop=False)
                    nc.tensor.matmul(m_ps[:, 0:L], lhsT=identb[:],
                                     rhs=ysn_all[:, k * B + j0:(k + 1) * B],
                                     start=False, stop=True)
                    etile = gp.tile([128, B], bf16, tag="g")
                    nc.scalar.activation(etile[:, 0:L], m_ps[:, 0:L], Act.Exp,
                                         scale=-2.0,
                                         bias=nsas[c][:, k:k + 1],
                                         accum_out=fsbs[c][:, k:k + 1])
                    etiles.append(etile)
                if c < 3:
                    coll_ps = cpsp.tile([KC, 384], f32, tag="coll",
                                        name=f"coll_ps{c}")
                    for k in range(KC):
                        nc.tensor.matmul(
                            coll_ps[:, 0:L - 128],
                            lhsT=kcols[:, k * KC:(k + 1) * KC],
                            rhs=etiles[k][:, 128:L],
                            start=(k == 0), stop=(k == KC - 1),
                            skip_group_check=True)
                    nc.scalar.activation(colls[c][:], coll_ps[:, 0:384 - 128 * c],
                                         Act.Copy)

            # ---- assemble fout: rowsums + bias-1 + transposed colsum pieces ----
            for c in range(NB):
                of = gp.tile([128, KC], f32, tag="of")
                nc.vector.tensor_add(of[:], fsbs[c][:], bbias[:])
                if c > 0:
                    # colsum pieces for output chunk c from collectors c' < c
                    acc16 = stagep.tile([KC, 128], f32, tag="acc16")
                    first = True
                    for cp in range(c):
                        off = 128 * (c - cp - 1)
                        piece = colls[cp][:, off:off + 128]
                        if first:
                            nc.vector.tensor_copy(acc16[:], piece)
                            first = False
                        else:
                            nc.vector.tensor_add(acc16[:], acc16[:], piece)
                    tp_ps = psp.tile([128, KC], f32, tag="ph1")
                    nc.tensor.transpose(tp_ps[:], acc16[:], identf[:])
                    nc.vector.tensor_add(of[:], of[:], tp_ps[:])
                nc.sync.dma_start(out=fout[c * 128:(c + 1) * 128, :], in_=of[:])

    nc.compile()
    return nc


def _get_program():
    if "nc" not in _cache:
        _cache["nc"] = _build()
    return _cache["nc"]


def kernel(x, theta, log_weight_scale, bias, _trace=False):
    import ml_dtypes
    from concourse.bass_utils import run_bass_kernel_spmd

    x = np.asarray(x, dtype=np.float32)
    theta = np.asarray(theta, dtype=np.float32)
    log_weight_scale = np.asarray(log_weight_scale, dtype=np.float32)
    bias = np.asarray(bias, dtype=np.float32)

    nc = _get_program()

    bf = ml_dtypes.bfloat16
    xTl = np.ascontiguousarray(
        x.T.reshape(ND, 128, B).transpose(1, 0, 2).reshape(128, ND * B)
    ).astype(bf)
    ident = np.eye(128, dtype=np.float32).astype(bf)
    # block selector: row (k,p) -> column k (for per-k sums over p)
    psel = np.repeat(np.eye(KC, dtype=np.float32), P, axis=0).astype(bf)
    # one-hot-row colsum weights: kcols[:, k*KC+m] = (m == k)
    kc_np = np.zeros((128, KC * KC), dtype=np.float32)
    for k_ in range(KC):
        kc_np[:, k_ * KC + k_] = 1.0
    kc_np = kc_np.astype(bf)

    in_maps = []
    for c in range(N_CORES):
        ks = slice(c * KC, (c + 1) * KC)
        th = np.ascontiguousarray(
            theta[:, ks, :].reshape(ND, 128, BT)
            .transpose(1, 0, 2).reshape(128, ND * BT)).astype(bf)
        lw = np.ascontiguousarray(
            log_weight_scale[ks, :].reshape(1, BT)).astype(np.float32)
        bi = np.ascontiguousarray(bias[ks].reshape(1, KC)).astype(np.float32)
        in_maps.append({"xT": xTl, "theta": th, "lws": lw, "biasc": bi,
                        "identin": ident, "pselin": psel, "kcolsin": kc_np})

    res = run_bass_kernel_spmd(nc, in_maps, list(range(N_CORES)),
                               trace=bool(_trace))
    f = np.concatenate([res.results[c]["fout"] for c in range(N_CORES)], axis=1)
    out = np.concatenate([x, f.astype(np.float32)], axis=1)
    if _trace:
        return out, res
    return out
